# revision 1
# baseline (speedup 1.0000x reference)
"""GAT encoder (2-layer GATConv + BatchNorm + ELU) on 8 Trainium2 NeuronCores.

Sharding: nodes are partitioned across the 8 cores (graph/data parallel).
Each core computes h = x @ W1 for its node range (plus attention logits via
appended projection columns), the h table is AllGathered (bf16), and each
core then runs the destination-sharded segment softmax + aggregation for its
node range using per-edge indirect-DMA gathers and one-hot selection matmuls.
BatchNorm statistics are AllReduced; layer 2 repeats the pattern at width 30.
"""
import sys
import numpy as np

try:
    import concourse.bacc as bacc
except ImportError:
    sys.path.insert(0, "/opt/trn_rl_repo")
    import concourse.bacc as bacc

import concourse.bass as bass
import concourse.tile as tile
import concourse.mybir as mybir
import concourse.bass_utils as bass_utils
from concourse.tile_rust import add_dep_helper

F32 = mybir.dt.float32
BF16 = mybir.dt.bfloat16
I32 = mybir.dt.int32
AX = mybir.AluOpType
ACT = mybir.ActivationFunctionType

P = 128
NCORES = 8
NEG_SLOPE = 0.2
BN_EPS = 1e-5


def _ceil_div(a, b):
    return (a + b - 1) // b


# ----------------------------------------------------------------------------
# Host-side edge preprocessing (index manipulation only)
# ----------------------------------------------------------------------------
def _prep_edges(edge_index, n_nodes):
    src = np.asarray(edge_index[0], dtype=np.int64)
    dst = np.asarray(edge_index[1], dtype=np.int64)
    # self loops, as in the reference
    loops = np.arange(n_nodes, dtype=np.int64)
    src = np.concatenate([src, loops])
    dst = np.concatenate([dst, loops])
    order = np.argsort(dst, kind="stable")
    src, dst = src[order], dst[order]

    nsh = n_nodes // NCORES
    nblk = _ceil_div(nsh, P)
    # split per core / per block
    core_of = dst // nsh
    starts = np.searchsorted(dst, np.arange(0, n_nodes + 1, 1))  # not used directly

    # edges per (core, block)
    blk_of = (dst % nsh) // P
    counts = np.zeros((NCORES, nblk), dtype=np.int64)
    for c in range(NCORES):
        m = core_of == c
        bb, cc = np.unique(blk_of[m], return_counts=True)
        counts[c, bb.astype(np.int64)] = cc
    chunks_per_block = np.maximum(1, _ceil_div(counts.max(axis=0), P))  # [nblk]
    tc_total = int(chunks_per_block.sum())

    # per-core flat tables [tc_total*P], padded with dst_local=-1
    src_t = np.zeros((NCORES, tc_total * P), dtype=np.int32)
    dstg_t = np.zeros((NCORES, tc_total * P), dtype=np.int32)
    dstl_t = np.full((NCORES, tc_total * P), -1.0, dtype=np.float32)
    # boundaries of each core's edge span
    core_starts = np.searchsorted(dst, np.arange(0, n_nodes + 1, nsh))
    for c in range(NCORES):
        e0, e1 = core_starts[c], core_starts[c + 1]
        s_c, d_c, b_c = src[e0:e1], dst[e0:e1], blk_of[e0:e1]
        blk_starts = np.searchsorted(b_c, np.arange(nblk + 1))
        pos = 0
        for b in range(nblk):
            i0, i1 = blk_starts[b], blk_starts[b + 1]
            cnt = i1 - i0
            sl = slice(pos * P, pos * P + cnt)
            src_t[c, sl] = s_c[i0:i1]
            dstg_t[c, sl] = d_c[i0:i1]
            dstl_t[c, sl] = (d_c[i0:i1] - c * nsh - b * P).astype(np.float32)
            pos += int(chunks_per_block[b])
        assert pos == tc_total
    # transpose to [P, tc_total] partition-major (partition = lane in chunk)
    src_t = src_t.reshape(NCORES, tc_total, P).transpose(0, 2, 1).copy()
    dstg_t = dstg_t.reshape(NCORES, tc_total, P).transpose(0, 2, 1).copy()
    dstl_t = dstl_t.reshape(NCORES, tc_total, P).transpose(0, 2, 1).copy()
    return chunks_per_block, src_t, dstg_t, dstl_t


# ----------------------------------------------------------------------------
# Device program
# ----------------------------------------------------------------------------
def _build(cfg):
    N = cfg["N"]; F = cfg["F"]; HID = cfg["HID"]; H = cfg["H"]; LAT = cfg["LAT"]
    nsh = N // NCORES
    nblk = _ceil_div(nsh, P)
    ntile = nblk
    nsh_pad = nblk * P
    HC = H * HID                     # 2048
    cpb = cfg["chunks_per_block"]    # list[int]
    TC = int(sum(cpb))
    fch = [(o, min(P, F - o)) for o in range(0, F, P)]   # F chunks (off, size)
    ncg = _ceil_div(HC, 512)         # col groups of 512 in HC
    kh2 = _ceil_div(HID, P)          # k-chunks for layer-2 matmul (4)
    L2W = LAT + 2                    # 32: [W2 | v2src | v2dst]
    replica = [list(range(NCORES))]

    nc = bacc.Bacc("TRN2", target_bir_lowering=False, debug=False,
                   enable_asserts=True, num_devices=NCORES)

    # ---- external inputs -------------------------------------------------
    x_in = nc.dram_tensor("x_shard", [nsh_pad, F], F32, kind="ExternalInput").ap()
    w1_in = nc.dram_tensor("W1", [F, HC], F32, kind="ExternalInput").ap()
    attsrc_mat_in = nc.dram_tensor("att_src_mat", [P, HC], F32, kind="ExternalInput").ap()
    attdst_mat_in = nc.dram_tensor("att_dst_mat", [P, HC], F32, kind="ExternalInput").ap()
    b1_mat_in = nc.dram_tensor("b1_mat", [P, HID], F32, kind="ExternalInput").ap()
    gamma_pg_in = nc.dram_tensor("gamma_pg", [P, HID // P], F32, kind="ExternalInput").ap()
    beta_pg_in = nc.dram_tensor("beta_pg", [P, HID // P], F32, kind="ExternalInput").ap()
    w2_in = nc.dram_tensor("W2", [HID, LAT], F32, kind="ExternalInput").ap()
    att2src_mat_in = nc.dram_tensor("att2_src_mat", [P, LAT], F32, kind="ExternalInput").ap()
    att2dst_mat_in = nc.dram_tensor("att2_dst_mat", [P, LAT], F32, kind="ExternalInput").ap()
    b2_mat_in = nc.dram_tensor("b2_mat", [P, LAT], F32, kind="ExternalInput").ap()
    srcidx_in = nc.dram_tensor("src_idx", [P, TC], I32, kind="ExternalInput").ap()
    dstgidx_in = nc.dram_tensor("dstg_idx", [P, TC], I32, kind="ExternalInput").ap()
    dstloc_in = nc.dram_tensor("dst_loc", [P, TC], F32, kind="ExternalInput").ap()

    z_out = nc.dram_tensor("z_shard", [nsh, LAT], F32, kind="ExternalOutput").ap()

    inv_n = 1.0 / N

    with tile.TileContext(nc) as tc:
        with tc.tile_pool(name="dramg", bufs=1, space="DRAM") as dramg, \
             tc.tile_pool(name="const", bufs=1) as const:
            # ---- persistent DRAM scratch ----
            h_shard = dramg.tile([nsh, HC], BF16)
            aux_shard = dramg.tile([nsh, 2 * H], F32)
            out1_dram = dramg.tile([nsh_pad, HID], F32)
            h2_shard = dramg.tile([nsh, L2W], F32)
            stats_bounce = dramg.tile([P, 2 * (HID // P)], F32)
            bar1_in = dramg.tile([1, 8], F32)
            bar2_in = dramg.tile([1, 8], F32)
            ssrow_dram = dramg.tile([P, 2 * (HID // P)], F32)

            # ---- persistent SBUF constants ----
            iota_row = const.tile([P, P], F32)
            nc.gpsimd.iota(iota_row[:], pattern=[[1, P]], base=0, channel_multiplier=0,
                           allow_small_or_imprecise_dtypes=True)
            ones_col = const.tile([1, P], F32)
            nc.vector.memset(ones_col[:], 1.0)
            srcidx_sb = const.tile([P, TC], I32)
            nc.sync.dma_start(srcidx_sb[:], srcidx_in[:])
            dstgidx_sb = const.tile([P, TC], I32)
            nc.sync.dma_start(dstgidx_sb[:], dstgidx_in[:])
            dstloc_sb = const.tile([P, TC], F32)
            nc.sync.dma_start(dstloc_sb[:], dstloc_in[:])
            wones_bf = const.tile([P, 1], BF16)
            nc.vector.memset(wones_bf[:], 1.0)
            wones_f32 = const.tile([P, 1], F32)
            nc.vector.memset(wones_f32[:], 1.0)

            upto = cfg.get("upto", 6)
            for _rep in range(cfg.get("repeat", 1)):
                # Shared collective outputs: single-writer constraint -> per rep
                h_full = dramg.tile([N, HC], BF16, addr_space="Shared", tag="h_full")
                aux_full = dramg.tile([N, 2 * H], F32, addr_space="Shared", tag="aux_full")
                h2_full = dramg.tile([N, L2W], F32, addr_space="Shared", tag="h2_full")
                stats_full = dramg.tile([P, 2 * (HID // P)], F32, addr_space="Shared", tag="stats_full")
                bar1_out = dramg.tile([1, 8], F32, addr_space="Shared", tag="bar1_out")
                bar2_out = dramg.tile([1, 8], F32, addr_space="Shared", tag="bar2_out")
                    # =========================================================
                    # Stage 0+1: h_shard = x @ W1 (bf16), aux = x @ [v_src|v_dst]
                    # =========================================================
                if upto >= 1:
                    with tc.tile_pool(name="s1w", bufs=1) as s1w, \
                         tc.tile_pool(name="s1", bufs=2) as s1, \
                         tc.tile_pool(name="s1p", bufs=2, space="PSUM") as s1p, \
                         tc.tile_pool(name="s1ph", bufs=1, space="PSUM") as s1ph:
                        # --- load + cast W1, build v vectors ---
                        w1bf = []
                        vabf = []
                        attsrc_mat = s1w.tile([P, HC], F32, tag="attm", bufs=1)
                        nc.sync.dma_start(attsrc_mat[:], attsrc_mat_in[:])
                        attdst_mat = s1w.tile([P, HC], F32, tag="attm2", bufs=1)
                        nc.sync.dma_start(attdst_mat[:], attdst_mat_in[:])
                        for ki, (ko, ks) in enumerate(fch):
                            w1f = s1.tile([P, HC], F32, tag="w1f")
                            nc.sync.dma_start(w1f[:ks, :], w1_in[ko:ko + ks, :])
                            wb = s1w.tile([P, HC], BF16, tag=f"w1bf{ki}")
                            nc.scalar.copy(wb[:ks, :], w1f[:ks, :])
                            w1bf.append(wb)
                            va = s1w.tile([P, 2 * H], F32, tag=f"va{ki}")
                            tmp = s1.tile([P, HC], F32, tag="vtmp")
                            nc.vector.tensor_tensor(out=tmp[:ks, :], in0=w1f[:ks, :],
                                                    in1=attsrc_mat[:ks, :], op=AX.mult)
                            for h in range(H):
                                nc.vector.reduce_sum(out=va[:ks, h:h + 1],
                                                     in_=tmp[:ks, h * HID:(h + 1) * HID],
                                                     axis=mybir.AxisListType.X)
                            nc.vector.tensor_tensor(out=tmp[:ks, :], in0=w1f[:ks, :],
                                                    in1=attdst_mat[:ks, :], op=AX.mult)
                            for h in range(H):
                                nc.vector.reduce_sum(out=va[:ks, H + h:H + h + 1],
                                                     in_=tmp[:ks, h * HID:(h + 1) * HID],
                                                     axis=mybir.AxisListType.X)
                            vabf.append(va)

                        ident = s1w.tile([P, P], F32, tag="ident", bufs=1)
                        from concourse.masks import make_identity
                        make_identity(nc, ident[:])

                        # --- main loop over node tiles ---
                        for nt in range(ntile):
                            r0 = nt * P
                            rows = min(P, nsh - r0)
                            x_t = s1.tile([P, F], F32, tag="xt")
                            nc.sync.dma_start(x_t[:], x_in[r0:r0 + P, :])
                            psum_h = s1ph.tile([P, HC], F32, space="PSUM", tag="ph")
                            psum_a = s1p.tile([P, 2 * H], F32, space="PSUM", tag="pa")
                            for ki, (ko, ks) in enumerate(fch):
                                pt = s1p.tile([P, P], F32, space="PSUM", tag="ptr")
                                nc.tensor.transpose(out=pt[:ks, :], in_=x_t[:, ko:ko + ks],
                                                    identity=ident[:])
                                xT_bf = s1.tile([P, P], BF16, tag="xtbf")
                                nc.scalar.copy(xT_bf[:ks, :], pt[:ks, :])
                                xT_f = s1.tile([P, P], F32, tag="xtf")
                                nc.vector.tensor_copy(out=xT_f[:ks, :], in_=pt[:ks, :])
                                first, last = ki == 0, ki == len(fch) - 1
                                for g in range(ncg):
                                    nc.tensor.matmul(
                                        out=psum_h[:, g * 512:(g + 1) * 512],
                                        lhsT=xT_bf[:ks, :],
                                        rhs=w1bf[ki][:ks, g * 512:(g + 1) * 512],
                                        start=first, stop=last)
                                nc.tensor.matmul(out=psum_a[:], lhsT=xT_f[:ks, :],
                                                 rhs=vabf[ki][:ks, :], start=first, stop=last)
                            hb = s1.tile([P, HC], BF16, tag="hb")
                            nc.scalar.copy(hb[:rows, :], psum_h[:rows, :])
                            nc.sync.dma_start(h_shard[r0:r0 + rows, :], hb[:rows, :])
                            ab = s1.tile([P, 2 * H], F32, tag="ab")
                            nc.vector.tensor_copy(out=ab[:rows, :], in_=psum_a[:rows, :])
                            nc.sync.dma_start(aux_shard[r0:r0 + rows, :], ab[:rows, :])

                    # =========================================================
                    # Stage 2: AllGather h + aux, barrier
                    # =========================================================
                if upto >= 2:
                    nc.gpsimd.collective_compute(
                        "AllGather", AX.bypass, replica_groups=replica,
                        ins=[h_shard[:]], outs=[h_full[:]])
                    nc.gpsimd.collective_compute(
                        "AllGather", AX.bypass, replica_groups=replica,
                        ins=[aux_shard[:]], outs=[aux_full[:]])
                    nc.gpsimd.dma_start(bar1_in[:], x_in[:1, :8])
                    bar1 = nc.gpsimd.collective_compute(
                        "AllReduce", AX.add, replica_groups=replica,
                        ins=[bar1_in[:]], outs=[bar1_out[:]])

                    # =========================================================
                    # Stage 3: layer-1 aggregation for the local dst shard
                    # =========================================================
                if upto >= 3:
                    stats_sb = const.tile([P, 2 * (HID // P)], F32)
                    nc.vector.memset(stats_sb[:], 0.0)
                    with tc.tile_pool(name="s3", bufs=3) as s3, \
                         tc.tile_pool(name="s3b", bufs=2) as s3b, \
                         tc.tile_pool(name="s3p", bufs=2, space="PSUM") as s3p, \
                         tc.tile_pool(name="s3pd", bufs=2, space="PSUM") as s3pd, \
                         tc.tile_pool(name="s3ph", bufs=1, space="PSUM") as s3ph:
                        b1m = s3b.tile([P, HID], F32, tag="b1m", bufs=1)
                        nc.sync.dma_start(b1m[:], b1_mat_in[:])
                        ch0 = 0
                        for b in range(nblk):
                            nchunks = int(cpb[b])
                            psum_o = s3ph.tile([P, HC], F32, space="PSUM", tag="po")
                            psum_d = s3pd.tile([P, H], F32, space="PSUM", tag="pd")
                            for ci in range(nchunks):
                                ch = ch0 + ci
                                g_t = s3.tile([P, HC], BF16, tag="g")
                                gi = nc.gpsimd.indirect_dma_start(
                                    out=g_t[:], out_offset=None, in_=h_full[:],
                                    in_offset=bass.IndirectOffsetOnAxis(
                                        ap=srcidx_sb[:, ch:ch + 1], axis=0))
                                add_dep_helper(gi.ins, bar1.ins, True, "gather after AG barrier")
                                asrc_t = s3.tile([P, H], F32, tag="asrc")
                                gi2 = nc.gpsimd.indirect_dma_start(
                                    out=asrc_t[:], out_offset=None, in_=aux_full[:],
                                    in_offset=bass.IndirectOffsetOnAxis(
                                        ap=srcidx_sb[:, ch:ch + 1], axis=0))
                                add_dep_helper(gi2.ins, bar1.ins, True, "gather after AG barrier")
                                adst_t = s3.tile([P, H], F32, tag="adst")
                                gi3 = nc.gpsimd.indirect_dma_start(
                                    out=adst_t[:], out_offset=None, in_=aux_full[:],
                                    in_offset=bass.IndirectOffsetOnAxis(
                                        ap=dstgidx_sb[:, ch:ch + 1], axis=0),
                                    element_offset=H)
                                add_dep_helper(gi3.ins, bar1.ins, True, "gather after AG barrier")
                                # one-hot (bf16 0/1): onehotT[j, i] = (dst_local[j] == i)
                                oht = s3.tile([P, P], BF16, tag="oht")
                                nc.vector.tensor_tensor(
                                    out=oht[:], in0=dstloc_sb[:, ch:ch + 1].to_broadcast([P, P]),
                                    in1=iota_row[:], op=AX.is_equal)
                                # e = lrelu(asrc + adst); w = exp(e)
                                e_t = s3.tile([P, H], F32, tag="e")
                                nc.vector.tensor_tensor(out=e_t[:], in0=asrc_t[:],
                                                        in1=adst_t[:], op=AX.add)
                                e2_t = s3.tile([P, H], F32, tag="e2")
                                nc.vector.tensor_scalar_mul(e2_t[:], e_t[:], NEG_SLOPE)
                                nc.vector.tensor_tensor(out=e_t[:], in0=e_t[:], in1=e2_t[:],
                                                        op=AX.max)
                                w_t = s3.tile([P, H], F32, tag="w")
                                nc.scalar.activation(w_t[:], e_t[:], ACT.Exp)
                                wbf_t = s3.tile([P, H], BF16, tag="wbf")
                                nc.vector.tensor_copy(out=wbf_t[:], in_=w_t[:])
                                first, last = ci == 0, ci == nchunks - 1
                                # denominator: one matmul, unscaled one-hot
                                nc.tensor.matmul(out=psum_d[:], lhsT=oht[:], rhs=wbf_t[:],
                                                 start=first, stop=last)
                                for h in range(H):
                                    lh = s3.tile([P, P], BF16, tag=f"lh{h}")
                                    if h % 2 == 0:
                                        nc.scalar.mul(lh[:], oht[:], w_t[:, h:h + 1])
                                    else:
                                        nc.vector.tensor_scalar_mul(lh[:], oht[:], w_t[:, h:h + 1])
                                    nc.tensor.matmul(
                                        out=psum_o[:, h * HID:(h + 1) * HID],
                                        lhsT=lh[:], rhs=g_t[:, h * HID:(h + 1) * HID],
                                        start=first, stop=last)
                            ch0 += nchunks
                            # --- combine heads: out = sum_h psum_o_h * 0.25/den_h + b1
                            den = s3b.tile([P, H], F32, tag="den")
                            nc.vector.tensor_scalar_add(den[:], psum_d[:], 1e-16)
                            rden = s3b.tile([P, H], F32, tag="rden")
                            nc.vector.reciprocal(rden[:], den[:])
                            nc.vector.tensor_scalar_mul(rden[:], rden[:], 1.0 / H)
                            ob = s3b.tile([P, HID], F32, tag="ob")
                            nc.scalar.mul(ob[:], psum_o[:, 0:HID], rden[:, 0:1])
                            for h in range(1, H):
                                tmph = s3b.tile([P, HID], F32, tag="tmph")
                                nc.scalar.mul(tmph[:], psum_o[:, h * HID:(h + 1) * HID],
                                              rden[:, h:h + 1])
                                nc.vector.tensor_tensor(out=ob[:], in0=ob[:], in1=tmph[:],
                                                        op=AX.add)
                            nc.vector.tensor_tensor(out=ob[:], in0=ob[:], in1=b1m[:], op=AX.add)
                            rows = min(P, nsh - b * P)
                            if rows < P:
                                # zero pad rows (p >= rows): keep where (rows-1-p) >= 0
                                nc.gpsimd.affine_select(
                                    out=ob[:], in_=ob[:], pattern=[[0, HID]],
                                    compare_op=AX.is_ge, fill=0.0,
                                    base=rows - 1, channel_multiplier=-1)
                            nc.sync.dma_start(out1_dram[b * P:(b + 1) * P, :], ob[:])
                            # --- BN statistics (block sums via matmul), accumulate in SBUF
                            sq = s3b.tile([P, HID], F32, tag="sq")
                            nc.scalar.square(sq[:], ob[:])
                            psum_s = s3pd.tile([P, 2 * (HID // P)], F32, space="PSUM", tag="ps")
                            for g in range(HID // P):
                                nc.tensor.matmul(out=psum_s[:, g:g + 1],
                                                 lhsT=ob[:, g * P:(g + 1) * P],
                                                 rhs=wones_f32[:], start=True, stop=True)
                                nc.tensor.matmul(out=psum_s[:, HID // P + g:HID // P + g + 1],
                                                 lhsT=sq[:, g * P:(g + 1) * P],
                                                 rhs=wones_f32[:], start=True, stop=True)
                            nc.vector.tensor_tensor(out=stats_sb[:], in0=stats_sb[:],
                                                    in1=psum_s[:], op=AX.add)

                    # =========================================================
                    # Stage 4: BN reduce + apply + ELU + h2 = x2 @ W2aug
                    # =========================================================
                if upto >= 4:
                    nc.gpsimd.dma_start(stats_bounce[:], stats_sb[:])
                    nc.gpsimd.collective_compute(
                        "AllReduce", AX.add, replica_groups=replica,
                        ins=[stats_bounce[:]], outs=[stats_full[:]])
                    NG = HID // P
                    with tc.tile_pool(name="s4c", bufs=1) as s4c, \
                         tc.tile_pool(name="s4", bufs=2) as s4, \
                         tc.tile_pool(name="s4p", bufs=2, space="PSUM") as s4p:
                        stf = s4c.tile([P, 2 * NG], F32)
                        nc.sync.dma_start(stf[:], stats_full[:])
                        mu = s4c.tile([P, NG], F32)
                        nc.vector.tensor_scalar_mul(mu[:], stf[:, 0:NG], inv_n)
                        ex2 = s4c.tile([P, NG], F32)
                        nc.vector.tensor_scalar_mul(ex2[:], stf[:, NG:2 * NG], inv_n)
                        var = s4c.tile([P, NG], F32)
                        nc.scalar.square(var[:], mu[:])
                        nc.vector.tensor_tensor(out=var[:], in0=ex2[:], in1=var[:], op=AX.subtract)
                        nc.vector.tensor_scalar_add(var[:], var[:], BN_EPS)
                        sd = s4c.tile([P, NG], F32)
                        nc.scalar.sqrt(sd[:], var[:])
                        rstd = s4c.tile([P, NG], F32)
                        nc.vector.reciprocal(rstd[:], sd[:])
                        gpg = s4c.tile([P, NG], F32)
                        nc.sync.dma_start(gpg[:], gamma_pg_in[:])
                        bpg = s4c.tile([P, NG], F32)
                        nc.sync.dma_start(bpg[:], beta_pg_in[:])
                        scal = s4c.tile([P, NG], F32)
                        nc.vector.tensor_tensor(out=scal[:], in0=gpg[:], in1=rstd[:], op=AX.mult)
                        shif = s4c.tile([P, NG], F32)
                        nc.vector.tensor_tensor(out=shif[:], in0=mu[:], in1=scal[:], op=AX.mult)
                        nc.vector.tensor_tensor(out=shif[:], in0=bpg[:], in1=shif[:], op=AX.subtract)
                        # DRAM roundtrip to get [1, HID] channel-major rows, then PE-broadcast
                        ssrow = s4c.tile([P, 2 * NG], F32)
                        nc.vector.tensor_copy(out=ssrow[:, 0:NG], in_=scal[:])
                        nc.vector.tensor_copy(out=ssrow[:, NG:2 * NG], in_=shif[:])
                        nc.sync.dma_start(ssrow_dram[:], ssrow[:])
                        scrow = s4c.tile([1, HID], F32)
                        src_ap = bass.AP(ssrow_dram.tensor, 0, [[1, NG], [2 * NG, P]])
                        nc.sync.dma_start(scrow[:1, :], src_ap)
                        shrow = s4c.tile([1, HID], F32)
                        src_ap2 = bass.AP(ssrow_dram.tensor, NG, [[1, NG], [2 * NG, P]])
                        nc.sync.dma_start(shrow[:1, :], src_ap2)
                        pb = s4p.tile([P, HID], F32, space="PSUM", tag="pb")
                        nc.tensor.matmul(out=pb[:], lhsT=ones_col[:], rhs=scrow[:1, :],
                                         start=True, stop=True)
                        scmat = s4c.tile([P, HID], F32)
                        nc.vector.tensor_copy(out=scmat[:], in_=pb[:])
                        pb2 = s4p.tile([P, HID], F32, space="PSUM", tag="pb")
                        nc.tensor.matmul(out=pb2[:], lhsT=ones_col[:], rhs=shrow[:1, :],
                                         start=True, stop=True)
                        shmat = s4c.tile([P, HID], F32)
                        nc.vector.tensor_copy(out=shmat[:], in_=pb2[:])

                        # W2aug
                        w2aug = s4c.tile([P, kh2, L2W], F32)
                        att2s = s4c.tile([P, LAT], F32)
                        nc.sync.dma_start(att2s[:], att2src_mat_in[:])
                        att2d = s4c.tile([P, LAT], F32)
                        nc.sync.dma_start(att2d[:], att2dst_mat_in[:])
                        for k in range(kh2):
                            nc.sync.dma_start(w2aug[:, k, 0:LAT], w2_in[k * P:(k + 1) * P, :])
                            t2 = s4.tile([P, LAT], F32, tag="t2")
                            nc.vector.tensor_tensor(out=t2[:], in0=w2aug[:, k, 0:LAT],
                                                    in1=att2s[:], op=AX.mult)
                            nc.vector.reduce_sum(out=w2aug[:, k, LAT:LAT + 1], in_=t2[:],
                                                 axis=mybir.AxisListType.X)
                            nc.vector.tensor_tensor(out=t2[:], in0=w2aug[:, k, 0:LAT],
                                                    in1=att2d[:], op=AX.mult)
                            nc.vector.reduce_sum(out=w2aug[:, k, LAT + 1:LAT + 2], in_=t2[:],
                                                 axis=mybir.AxisListType.X)
                        ident2 = s4c.tile([P, P], F32)
                        from concourse.masks import make_identity as _mi
                        _mi(nc, ident2[:])

                        for nt in range(ntile):
                            r0 = nt * P
                            rows = min(P, nsh - r0)
                            o1 = s4.tile([P, HID], F32, tag="o1")
                            nc.sync.dma_start(o1[:], out1_dram[r0:r0 + P, :])
                            x2 = s4.tile([P, HID], F32, tag="x2")
                            nc.vector.tensor_tensor(out=x2[:], in0=o1[:], in1=scmat[:], op=AX.mult)
                            nc.vector.tensor_tensor(out=x2[:], in0=x2[:], in1=shmat[:], op=AX.add)
                            # ELU = relu(x2) + exp(min(x2,0)) - 1
                            rl = s4.tile([P, HID], F32, tag="rl")
                            nc.scalar.activation(rl[:], x2[:], ACT.Relu)
                            mn = s4.tile([P, HID], F32, tag="mn")
                            nc.vector.tensor_scalar_min(mn[:], x2[:], 0.0)
                            em = s4.tile([P, HID], F32, tag="em")
                            nc.scalar.activation(em[:], mn[:], ACT.Exp)
                            nc.vector.tensor_scalar_add(em[:], em[:], -1.0)
                            nc.vector.tensor_tensor(out=x2[:], in0=rl[:], in1=em[:], op=AX.add)
                            # h2 = x2 @ W2aug
                            ph2 = s4p.tile([P, L2W], F32, space="PSUM", tag="ph2")
                            for k in range(kh2):
                                ptr = s4p.tile([P, P], F32, space="PSUM", tag="ptr2")
                                nc.tensor.transpose(out=ptr[:], in_=x2[:, k * P:(k + 1) * P],
                                                    identity=ident2[:])
                                x2T = s4.tile([P, P], F32, tag="x2T")
                                nc.vector.tensor_copy(out=x2T[:], in_=ptr[:])
                                nc.tensor.matmul(out=ph2[:], lhsT=x2T[:], rhs=w2aug[:, k, :],
                                                 start=(k == 0), stop=(k == kh2 - 1))
                            h2b = s4.tile([P, L2W], F32, tag="h2b")
                            nc.vector.tensor_copy(out=h2b[:], in_=ph2[:])
                            nc.sync.dma_start(h2_shard[r0:r0 + rows, :], h2b[:rows, :])

                    # =========================================================
                    # Stage 5: AllGather h2, barrier
                    # =========================================================
                if upto >= 5:
                    nc.gpsimd.collective_compute(
                        "AllGather", AX.bypass, replica_groups=replica,
                        ins=[h2_shard[:]], outs=[h2_full[:]])
                    nc.gpsimd.dma_start(bar2_in[:], x_in[:1, :8])
                    bar2 = nc.gpsimd.collective_compute(
                        "AllReduce", AX.add, replica_groups=replica,
                        ins=[bar2_in[:]], outs=[bar2_out[:]])

                    # =========================================================
                    # Stage 6: layer-2 aggregation
                    # =========================================================
                if upto >= 6:
                    with tc.tile_pool(name="s6", bufs=3) as s6, \
                         tc.tile_pool(name="s6b", bufs=2) as s6b, \
                         tc.tile_pool(name="s6p", bufs=2, space="PSUM") as s6p:
                        b2m = s6b.tile([P, LAT], F32, tag="b2m", bufs=1)
                        nc.sync.dma_start(b2m[:], b2_mat_in[:])
                        ch0 = 0
                        for b in range(nblk):
                            nchunks = int(cpb[b])
                            rows = min(P, nsh - b * P)
                            psum_o2 = s6p.tile([P, LAT], F32, space="PSUM", tag="po2")
                            psum_d2 = s6p.tile([P, 1], F32, space="PSUM", tag="pd2")
                            for ci in range(nchunks):
                                ch = ch0 + ci
                                g2 = s6.tile([P, L2W], F32, tag="g2")
                                gj = nc.gpsimd.indirect_dma_start(
                                    out=g2[:], out_offset=None, in_=h2_full[:],
                                    in_offset=bass.IndirectOffsetOnAxis(
                                        ap=srcidx_sb[:, ch:ch + 1], axis=0))
                                add_dep_helper(gj.ins, bar2.ins, True, "gather after AG barrier")
                                a2d = s6.tile([P, 1], F32, tag="a2d")
                                gj2 = nc.gpsimd.indirect_dma_start(
                                    out=a2d[:], out_offset=None, in_=h2_full[:],
                                    in_offset=bass.IndirectOffsetOnAxis(
                                        ap=dstgidx_sb[:, ch:ch + 1], axis=0),
                                    element_offset=L2W - 1)
                                add_dep_helper(gj2.ins, bar2.ins, True, "gather after AG barrier")
                                oht2 = s6.tile([P, P], F32, tag="oht2")
                                nc.vector.tensor_tensor(
                                    out=oht2[:], in0=dstloc_sb[:, ch:ch + 1].to_broadcast([P, P]),
                                    in1=iota_row[:], op=AX.is_equal)
                                e_t = s6.tile([P, 1], F32, tag="e1")
                                nc.vector.tensor_tensor(out=e_t[:], in0=g2[:, LAT:LAT + 1],
                                                        in1=a2d[:], op=AX.add)
                                e2_t = s6.tile([P, 1], F32, tag="e21")
                                nc.vector.tensor_scalar_mul(e2_t[:], e_t[:], NEG_SLOPE)
                                nc.vector.tensor_tensor(out=e_t[:], in0=e_t[:], in1=e2_t[:],
                                                        op=AX.max)
                                w_t = s6.tile([P, 1], F32, tag="w1t")
                                nc.scalar.activation(w_t[:], e_t[:], ACT.Exp)
                                lh2 = s6.tile([P, P], F32, tag="lh2")
                                nc.vector.tensor_scalar_mul(lh2[:], oht2[:], w_t[:, 0:1])
                                first, last = ci == 0, ci == nchunks - 1
                                nc.tensor.matmul(out=psum_d2[:], lhsT=lh2[:], rhs=wones_f32[:],
                                                 start=first, stop=last)
                                nc.tensor.matmul(out=psum_o2[:], lhsT=lh2[:], rhs=g2[:, 0:LAT],
                                                 start=first, stop=last)
                            ch0 += nchunks
                            den2 = s6b.tile([P, 1], F32, tag="den2")
                            nc.vector.tensor_scalar_add(den2[:], psum_d2[:], 1e-16)
                            rden2 = s6b.tile([P, 1], F32, tag="rden2")
                            nc.vector.reciprocal(rden2[:], den2[:])
                            zb = s6b.tile([P, LAT], F32, tag="zb")
                            nc.scalar.mul(zb[:], psum_o2[:], rden2[:, 0:1])
                            nc.vector.tensor_tensor(out=zb[:], in0=zb[:], in1=b2m[:], op=AX.add)
                            nc.sync.dma_start(z_out[b * P:b * P + rows, :], zb[:rows, :])

    nc.compile()
    return nc


# ----------------------------------------------------------------------------
# Public entry point
# ----------------------------------------------------------------------------
_CACHE = {}


def _run(inputs, trace=False, repeat=1, upto=6):
    x = np.ascontiguousarray(np.asarray(inputs["x"], dtype=np.float32))
    N, F = x.shape
    W1 = np.ascontiguousarray(np.asarray(inputs["W1"], dtype=np.float32))
    att_src1 = np.asarray(inputs["att_src1"], dtype=np.float32)
    att_dst1 = np.asarray(inputs["att_dst1"], dtype=np.float32)
    b1 = np.asarray(inputs["b1"], dtype=np.float32)
    gamma = np.asarray(inputs["gamma"], dtype=np.float32)
    beta = np.asarray(inputs["beta"], dtype=np.float32)
    W2 = np.ascontiguousarray(np.asarray(inputs["W2"], dtype=np.float32))
    att_src2 = np.asarray(inputs["att_src2"], dtype=np.float32)
    att_dst2 = np.asarray(inputs["att_dst2"], dtype=np.float32)
    b2 = np.asarray(inputs["b2"], dtype=np.float32)
    H, HID = att_src1.shape
    LAT = W2.shape[1]

    chunks_per_block, src_t, dstg_t, dstl_t = _prep_edges(inputs["edge_index"], N)

    cfg_key = (N, F, HID, H, LAT, repeat, upto, tuple(int(v) for v in chunks_per_block))
    if cfg_key not in _CACHE:
        cfg = dict(N=N, F=F, HID=HID, H=H, LAT=LAT, repeat=repeat, upto=upto,
                   chunks_per_block=[int(v) for v in chunks_per_block])
        _CACHE[cfg_key] = _build(cfg)
    nc = _CACHE[cfg_key]

    nsh = N // NCORES
    nblk = _ceil_div(nsh, P)
    nsh_pad = nblk * P

    att_src_mat = np.tile(att_src1.reshape(1, H * HID), (P, 1))
    att_dst_mat = np.tile(att_dst1.reshape(1, H * HID), (P, 1))
    b1_mat = np.tile(b1.reshape(1, HID), (P, 1))
    gamma_pg = gamma.reshape(HID // P, P).T.copy()
    beta_pg = beta.reshape(HID // P, P).T.copy()
    att2s_mat = np.tile(att_src2.reshape(1, LAT), (P, 1))
    att2d_mat = np.tile(att_dst2.reshape(1, LAT), (P, 1))
    b2_mat = np.tile(b2.reshape(1, LAT), (P, 1))

    in_maps = []
    for c in range(NCORES):
        xs = np.zeros((nsh_pad, F), dtype=np.float32)
        xs[:nsh] = x[c * nsh:(c + 1) * nsh]
        in_maps.append({
            "x_shard": xs, "W1": W1,
            "att_src_mat": att_src_mat, "att_dst_mat": att_dst_mat,
            "b1_mat": b1_mat, "gamma_pg": gamma_pg, "beta_pg": beta_pg,
            "W2": W2, "att2_src_mat": att2s_mat, "att2_dst_mat": att2d_mat,
            "b2_mat": b2_mat,
            "src_idx": src_t[c], "dstg_idx": dstg_t[c], "dst_loc": dstl_t[c],
        })
    last_err = None
    for attempt in range(3):
        try:
            res = bass_utils.run_bass_kernel_spmd(
                nc, in_maps, core_ids=list(range(NCORES)), trace=trace)
            break
        except Exception as e:  # transient NRT/axon failures: back off and retry
            last_err = e
            import time as _time
            _time.sleep(10 * (attempt + 1))
            try:
                import jax
                jax.clear_caches()
            except Exception:
                pass
    else:
        raise last_err
    z = np.concatenate([res.results[c]["z_shard"] for c in range(NCORES)], axis=0)
    return z.astype(np.float32), res


def kernel(**inputs) -> np.ndarray:
    z, _ = _run(inputs, trace=False)
    return z

